# revision 16
# baseline (speedup 1.0000x reference)
"""Distributed attention layer kernel for 8 TRN2 NeuronCores.

Math (per reference): out = softmax_causal((x@Wq)(x@Wk)^T / 8) @ (x@Wv) @ Wo
with B=4, S=2048, D=1024, H=16 heads of dim 64.

Sharding: head tensor-parallel. Core c owns head pair (2c, 2c+1):
  - Wq/Wk/Wv column-sharded [1024, 128]; statesT replicated [1024, 8192].
  - Each core computes qT/kT/vT for its 2 heads, causal attention in
    S^T layout (kj on partitions, qi on free), softmax denominator via a
    ones-column appended to V (PV matmul row 64 = sum of probs).
  - ctx^T tiles are normalized straight out of PSUM: the raw denominator
    row is partition-broadcast on GpSimd (idle engine), inverted in one
    reciprocal_approx_fast, and a single DVE mul (PSUM x SBUF -> bf16)
    writes the AllToAll staging tile. This keeps the in-order DVE queue
    shallow so diag mask-muls never stall PV matmuls.
  - Three AllToAlls: C0 = batches {0,1} (fires after b1, transfer hidden
    under b2's attention), C1 = batch {2} (hidden under b3), C2 = batch
    {3} (small 0.25MB tail, covered by proj(C1) as keep-warm work).
  - Output projection: out_rows = sum_c slab_c.T @ Wo[128c:...] in PSUM.
    proj(C0) interleaves into b3's attention as PE filler; proj(C1)
    drains right after the C2 trigger; proj(C2) is the only exposed tail.

Scheduling: the PE instruction stream is kept dense to hold the clock
at the top p-state. QKV for batch b+1 is interleaved as filler between
the score/PV matmul pairs of batch b's attention. Causal masking of
diagonal blocks is a post-exp multiply by a 0/1 triangular mask (bf16,
SBUF) so the score->exp chain never waits on a PSUM-side DVE add.

Matmul operands are bf16 (PE full rate); accumulation is fp32 in PSUM.
"""

import ml_dtypes
import numpy as np

import concourse.bass as bass
import concourse.mybir as mybir
import concourse.tile as tile
from concourse import bacc
from concourse.masks import make_identity

F32 = mybir.dt.float32
BF16 = mybir.dt.bfloat16

B, S, D, H = 4, 2048, 1024, 16
HD = 64
N_CORES = 8
QI, KJ = 512, 128

# output row chunks per collective: C0 = batches {0,1}, C1 = {2}, C2 = {3}
CHUNK_BATCHES = ((0, 1), (2,), (3,))


def build_tri(KJ=KJ):
    """tri[p, f] = 1.0 if p <= f else 0.0 (valid causal positions of the
    first KJ columns of a diagonal strip)."""
    p = np.arange(KJ)[:, None]
    f = np.arange(KJ)[None, :]
    return np.where(p <= f, 1.0, 0.0).astype(ml_dtypes.bfloat16)


def build(b_=B, s_=S, d_=D, n_cores=N_CORES):
    HPC = d_ // n_cores          # head cols per core (2 heads x 64)
    NH = HPC // HD               # heads per core (2)
    R = b_ * s_                  # global rows (8192)
    Rc = R // n_cores            # output rows per core (1024)
    DT = d_ // 128               # contraction tiles (8)
    SKJ = s_ // KJ               # kj blocks per (b, h) (16)
    SQI = s_ // QI               # qi tiles per (b, h) (4)
    CL = s_ // QI                # column tiles per batch (4)
    # rows per core per chunk
    CROWS = [len(cb) * s_ // n_cores for cb in CHUNK_BATCHES]  # 512, 256, 256
    assert s_ % QI == 0 and d_ % 128 == 0

    nc = bacc.Bacc(None, target_bir_lowering=False, debug=False)
    statesT = nc.declare_dram_parameter("statesT", [d_, R], BF16, isOutput=False)
    wq = nc.declare_dram_parameter("wq", [d_, HPC], BF16, isOutput=False)
    wk = nc.declare_dram_parameter("wk", [d_, HPC], BF16, isOutput=False)
    wv = nc.declare_dram_parameter("wv", [d_, HPC], BF16, isOutput=False)
    wo = nc.declare_dram_parameter("wo", [d_, d_], BF16, isOutput=False)
    tri_in = nc.declare_dram_parameter("tri", [KJ, KJ], BF16, isOutput=False)
    out_ext = nc.declare_dram_parameter("out", [Rc, d_], F32, isOutput=True)

    SC = float(1.0 / np.sqrt(HD))
    EXP = mybir.ActivationFunctionType.Exp

    with tile.TileContext(nc) as tc:
        with tc.tile_pool(name="persist", bufs=1) as pp, \
             tc.tile_pool(name="dram", bufs=1, space="DRAM") as dram:
            a2a_in = [dram.tile([n_cores * HPC, CROWS[i]], BF16,
                                tag=f"a2a_in{i}", name=f"a2a_in{i}")
                      for i in range(3)]
            a2a_out = [dram.tile([n_cores * HPC, CROWS[i]], BF16,
                                 tag=f"a2a_out{i}", name=f"a2a_out{i}")
                       for i in range(3)]

            qT = pp.tile([HPC, R], BF16, tag="qT")
            kT = pp.tile([HPC, R], BF16, tag="kT")
            vp = pp.tile([KJ, b_ * NH * SKJ, HD + 1], BF16, tag="vp")
            w_sb = pp.tile([128, 3, DT, HPC], BF16, tag="w_sb")
            wo_sb = pp.tile([128, DT, d_], BF16, tag="wo_sb")
            tri_sb = pp.tile([KJ, KJ], BF16, tag="tri_sb")
            ident = pp.tile([128, 128], BF16, tag="ident")

            with tc.tile_pool(name="st_in", bufs=4) as stp, \
                 tc.tile_pool(name="vT_pool", bufs=2) as vtp, \
                 tc.tile_pool(name="ps_ps", bufs=2, space="PSUM") as qps, \
                 tc.tile_pool(name="sp_ps", bufs=2, space="PSUM") as spp, \
                 tc.tile_pool(name="ctx_ps", bufs=2, space="PSUM") as cps, \
                 tc.tile_pool(name="pt_sb", bufs=6) as ptp, \
                 tc.tile_pool(name="ctxu_sb", bufs=4) as cup, \
                 tc.tile_pool(name="recip_sb", bufs=3) as rpp, \
                 tc.tile_pool(name="ctxT_sb", bufs=4) as ctp, \
                 tc.tile_pool(name="slab_sb", bufs=2) as slp, \
                 tc.tile_pool(name="o_sb", bufs=3) as osp:

                # ---- prologue: start input DMAs early
                st_tiles = {}

                def issue_st(ci):
                    st = stp.tile([128, DT, QI], BF16, tag="st", name="st")
                    for dd in range(DT):
                        nc.sync.dma_start(
                            out=st[:, dd],
                            in_=statesT[dd * 128:(dd + 1) * 128,
                                        ci * QI:(ci + 1) * QI])
                    st_tiles[ci] = st

                issue_st(0)
                issue_st(1)
                issue_st(2)
                nc.sync.dma_start(out=tri_sb[:], in_=tri_in[:, :])
                for i, w in enumerate([wq, wk, wv]):
                    nc.sync.dma_start(
                        out=w_sb[:, i], in_=w[:, :].rearrange("(t p) c -> p t c", p=128))
                nc.sync.dma_start(
                    out=wo_sb[:], in_=wo[:, :].rearrange("(t p) n -> p t n", p=128))
                make_identity(nc, ident[:])
                nc.vector.memset(vp[:, :, HD], 1.0)

                def qkv_units(bb):
                    """Yield-granular QKV + V' transposes for batch bb."""
                    vT = vtp.tile([HPC, s_], BF16, tag="vT", name="vT")
                    for cl in range(CL):
                        ci = bb * CL + cl
                        if ci + 3 < b_ * CL:
                            issue_st(ci + 3)
                        st = st_tiles.pop(ci)
                        yield
                        for pi, dest, off in ((2, vT, cl * QI), (0, qT, ci * QI),
                                              (1, kT, ci * QI)):
                            ps = qps.tile([128, QI], F32, tag="ps", name="ps")
                            for dd in range(DT):
                                nc.tensor.matmul(
                                    ps[:], w_sb[:, pi, dd], st[:, dd],
                                    start=(dd == 0), stop=(dd == DT - 1))
                                if dd % 2 == 1:
                                    yield
                            nc.vector.tensor_copy(dest[:, off:off + QI], ps[:])
                            yield
                        # h-inner: consecutive transposes load alternating
                        # row groups (base partition 0/64), so each LDW pulls
                        # ahead under the other head's in-flight transpose
                        for kj in range(cl * (SKJ // CL), (cl + 1) * (SKJ // CL)):
                            for h in range(NH):
                                blk = (bb * NH + h) * SKJ + kj
                                tp = qps.tile([KJ, HD], BF16, tag="ps", name="tp")
                                nc.tensor.transpose(
                                    tp[0:KJ, 0:HD],
                                    vT[h * HD:(h + 1) * HD, kj * KJ:(kj + 1) * KJ],
                                    ident[h * HD:(h + 1) * HD, h * HD:(h + 1) * HD])
                                nc.vector.tensor_copy(vp[:, blk, 0:HD],
                                                      tp[0:KJ, 0:HD])
                                yield

                def dma_ctx_to_a2a(ctxT, bb, qi, hh):
                    """Stage a normalized ctx^T tile into its chunk buffer."""
                    chunk = 0 if bb < 2 else bb - 1
                    r0 = bb * s_ + qi * QI            # global row of tile col 0
                    base = 0 if chunk == 0 else (4096 if chunk == 1 else 6144)
                    crows = CROWS[chunk]
                    # tile cols [c0, c0+crows) -> dest core j, chunk-col offset
                    for part in range(QI // crows):
                        c0 = part * crows
                        j = (r0 + c0 - base) // crows
                        nc.sync.dma_start(
                            out=a2a_in[chunk][j * HPC + hh * HD:
                                              j * HPC + (hh + 1) * HD, 0:crows],
                            in_=ctxT[:, c0:c0 + crows])

                def load_slab(chunk):
                    """Issue the slab DMAs for one chunk. Must be emitted at
                    a point where collective `chunk` is known complete: a
                    DMA descriptor waiting on its semaphore would sit at the
                    queue head and also because the next collective's ring
                    traffic is FIFO-ordered behind it in the same queues."""
                    crows = CROWS[chunk]
                    slab = slp.tile([HPC, n_cores, crows], BF16,
                                    tag=f"slab{chunk}", name="slab", bufs=1)
                    for c in range(n_cores):
                        nc.sync.dma_start(
                            out=slab[:, c],
                            in_=a2a_out[chunk][c * HPC:(c + 1) * HPC, :])
                    return slab

                def proj_units(chunk, slab=None):
                    """Yield-granular output projection for one chunk."""
                    crows = CROWS[chunk]
                    obase = sum(CROWS[:chunk])
                    if slab is None:
                        slab = load_slab(chunk)
                    for m in range(crows // 128):
                        for n in range(d_ // QI):
                            ps = qps.tile([128, QI], F32, tag="ps", name="ops")
                            for c in range(n_cores):
                                nc.tensor.matmul(
                                    ps[:],
                                    slab[:, c, m * 128:(m + 1) * 128],
                                    wo_sb[:, c, n * QI:(n + 1) * QI],
                                    start=(c == 0), stop=(c == n_cores - 1))
                                yield
                            ob = osp.tile([128, QI], F32, tag="ob", name="ob")
                            nc.vector.tensor_copy(ob[:], ps[:])
                            nc.sync.dma_start(
                                out=out_ext[obase + m * 128:
                                            obase + (m + 1) * 128,
                                            n * QI:(n + 1) * QI],
                                in_=ob[:])
                            yield

                # Two-stage epilogue pipeline, serviced once per qi. Stage A
                # pulls the raw denominator row + ctx rows to SBUF (freeing
                # the PSUM bank for the next qi) and kicks the GpSimd
                # partition-broadcast (its ~2us latency is hidden: stage B
                # runs a full qi later). Stage B inverts the broadcast
                # denominator in one fast DVE op and one bf16 mul writes the
                # AllToAll staging tile.
                stage_a = []
                stage_b = []

                def service():
                    while stage_b:
                        ctxu, rbd, bb, qi, hh = stage_b.pop(0)
                        rb = rpp.tile([HD, QI], F32, tag="rb", name="rb",
                                      bufs=4)
                        nc.vector.reciprocal_approx_fast(rb[:], rbd[:])
                        ctxT = ctp.tile([HD, QI], BF16, tag="ctxT",
                                        name="ctxT")
                        nc.vector.tensor_mul(ctxT[:], ctxu[:], rb[:])
                        dma_ctx_to_a2a(ctxT, bb, qi, hh)
                    while stage_a:
                        ctx, bb, qi, hh = stage_a.pop(0)
                        # den must land on partition 0: partition_broadcast
                        # broadcasts partition 0 regardless of the AP offset
                        den0 = rpp.tile([1, QI], F32, tag="den0",
                                        name="den0", bufs=4)
                        nc.vector.tensor_copy(den0[:], ctx[HD:HD + 1, :])
                        ctxu = cup.tile([HD, QI], BF16, tag="ctxu",
                                        name="ctxu", bufs=4)
                        nc.vector.tensor_copy(ctxu[:], ctx[0:HD, :])
                        rbd = rpp.tile([HD, QI], F32, tag="rbd", name="rbd",
                                       bufs=4)
                        nc.gpsimd.partition_broadcast(rbd[:], den0[:])
                        stage_b.append((ctxu, rbd, bb, qi, hh))

                def flush_pending():
                    service()
                    service()

                def attn_pair(bb, fill, pre_qi=None):
                    """Attention for both heads of batch bb, interleaved so
                    the two heads' K=64 score matmuls sit in adjacent row
                    groups of the PE array (tile_position auto-derived from
                    base partition 0/64) and run concurrently; their
                    LDWEIGHTS pull ahead into the other head's row group."""
                    base = bb * s_
                    for qi in range(SQI):
                        if pre_qi is not None:
                            pre_qi(qi)
                        # service point first: frees last qi's ctx banks
                        # (stage-A copies) before this qi's PVs need them
                        service()
                        ctxs = [cps.tile([HD + 1, QI], F32, tag="ctx",
                                         name="ctx") for _ in range(NH)]
                        q0 = base + qi * QI

                        def s_mm(h, out_ap, kj, coff):
                            nc.tensor.matmul(
                                out_ap,
                                kT[h * HD:(h + 1) * HD,
                                   base + kj * KJ: base + (kj + 1) * KJ],
                                qT[h * HD:(h + 1) * HD, q0 + coff: q0 + QI],
                                start=True, stop=True)

                        def pv_mm(h, kj, rhs_ap, coff, start, stop):
                            blk = (bb * NH + h) * SKJ + kj
                            nc.tensor.matmul(
                                ctxs[h][:, coff:QI], vp[:, blk], rhs_ap,
                                start=start, stop=stop)

                        # Diagonal strip: 4 blocks x 2 heads packed into 3
                        # PSUM regions / 3 exps. Layout per region (bank0 |
                        # bank1): A = h0·di0 | h1·di0; B = h0·di1 | h1·di1;
                        # C = h0·di2, h0·di3 | h1·di2, h1·di3.
                        dkj = SQI * qi
                        rA = spp.tile([128, 2 * QI], F32, tag="sp", name="rA")
                        s_mm(0, rA[:, 0:QI], dkj + 0, 0)
                        s_mm(1, rA[:, QI:2 * QI], dkj + 0, 0)
                        pA = ptp.tile([128, 2 * QI], BF16, tag="pt", name="pA")
                        nc.scalar.activation(pA[:], rA[:], EXP, scale=SC)
                        nc.vector.tensor_mul(pA[:, 0:KJ], pA[:, 0:KJ], tri_sb[:])
                        nc.vector.tensor_mul(pA[:, QI:QI + KJ],
                                             pA[:, QI:QI + KJ], tri_sb[:])
                        fill(2)
                        rB = spp.tile([128, 2 * QI], F32, tag="sp", name="rB")
                        s_mm(0, rB[:, 0:QI - KJ], dkj + 1, KJ)
                        s_mm(1, rB[:, QI:2 * QI - KJ], dkj + 1, KJ)
                        pv_mm(0, dkj + 0, pA[:, 0:QI], 0, True, False)
                        pv_mm(1, dkj + 0, pA[:, QI:2 * QI], 0, True, False)
                        fill(1)
                        pB = ptp.tile([128, 2 * QI], BF16, tag="pt", name="pB")
                        nc.scalar.activation(pB[:, 0:2 * QI - KJ],
                                             rB[:, 0:2 * QI - KJ], EXP,
                                             scale=SC)
                        nc.vector.tensor_mul(pB[:, 0:KJ], pB[:, 0:KJ], tri_sb[:])
                        nc.vector.tensor_mul(pB[:, QI:QI + KJ],
                                             pB[:, QI:QI + KJ], tri_sb[:])
                        fill(1)
                        rC = spp.tile([128, 2 * QI], F32, tag="sp", name="rC")
                        s_mm(0, rC[:, 0:2 * KJ], dkj + 2, 2 * KJ)
                        s_mm(1, rC[:, QI:QI + 2 * KJ], dkj + 2, 2 * KJ)
                        s_mm(0, rC[:, 2 * KJ:3 * KJ], dkj + 3, 3 * KJ)
                        s_mm(1, rC[:, QI + 2 * KJ:QI + 3 * KJ], dkj + 3, 3 * KJ)
                        pv_mm(0, dkj + 1, pB[:, 0:QI - KJ], KJ, False, False)
                        pv_mm(1, dkj + 1, pB[:, QI:2 * QI - KJ], KJ,
                              False, False)
                        fill(1)
                        pC = ptp.tile([128, 2 * QI], BF16, tag="pt", name="pC")
                        nc.scalar.activation(pC[:, 0:QI + 3 * KJ],
                                             rC[:, 0:QI + 3 * KJ], EXP,
                                             scale=SC)
                        for h in range(NH):
                            off = h * QI
                            nc.vector.tensor_mul(pC[:, off:off + KJ],
                                                 pC[:, off:off + KJ],
                                                 tri_sb[:])
                            nc.vector.tensor_mul(
                                pC[:, off + 2 * KJ:off + 3 * KJ],
                                pC[:, off + 2 * KJ:off + 3 * KJ], tri_sb[:])
                        fill(2)
                        last_diag = qi == 0
                        pv_mm(0, dkj + 2, pC[:, 0:2 * KJ], 2 * KJ,
                              False, False)
                        pv_mm(1, dkj + 2, pC[:, QI:QI + 2 * KJ], 2 * KJ,
                              False, False)
                        pv_mm(0, dkj + 3, pC[:, 2 * KJ:3 * KJ], 3 * KJ,
                              False, last_diag)
                        pv_mm(1, dkj + 3, pC[:, QI + 2 * KJ:QI + 3 * KJ],
                              3 * KJ, False, last_diag)
                        fill(1)
                        # full blocks: one kj x both heads per region
                        for kj in range(SQI * qi):
                            reg = spp.tile([128, 2 * QI], F32, tag="sp",
                                           name="reg")
                            s_mm(0, reg[:, 0:QI], kj, 0)
                            s_mm(1, reg[:, QI:2 * QI], kj, 0)
                            pt = ptp.tile([128, 2 * QI], BF16, tag="pt",
                                          name="pt")
                            nc.scalar.activation(pt[:], reg[:], EXP, scale=SC)
                            fill(2)
                            stop = kj == SQI * qi - 1
                            pv_mm(0, kj, pt[:, 0:QI], 0, False, stop)
                            pv_mm(1, kj, pt[:, QI:2 * QI], 0, False, stop)
                            fill(1)
                        for h in range(NH):
                            stage_a.append((ctxs[h], bb, qi, h))
                        fill(1)

                def drain(gen):
                    for _ in gen:
                        pass

                def make_fill(gen):
                    box = {"g": gen}

                    def fill(n=1):
                        g = box["g"]
                        if g is None:
                            return
                        for _ in range(n):
                            try:
                                next(g)
                            except StopIteration:
                                box["g"] = None
                                return
                    return fill, box

                slabs = {}

                def pre_qi_b3(qi):
                    # C1 completed early in b3's attention; issue its slab
                    # loads before C2's collective ring traffic (FIFO DMA
                    # queues) can get ahead of them.
                    if qi == SQI - 1:
                        slabs[1] = load_slab(1)

                drain(qkv_units(0))
                for bb in range(b_):
                    if bb < b_ - 1:
                        gen = qkv_units(bb + 1)
                    else:
                        # C0 completed during b2's attention; its slab is
                        # ready, so proj(C0) interleaves as real PE filler.
                        gen = proj_units(0)
                    fill, box = make_fill(gen)
                    attn_pair(bb, fill, pre_qi_b3 if bb == b_ - 1 else None)
                    if bb >= 1:
                        flush_pending()
                        i = bb - 1
                        nc.gpsimd.collective_compute(
                            "AllToAll", mybir.AluOpType.bypass,
                            replica_groups=[list(range(n_cores))],
                            ins=[a2a_in[i][:].opt()],
                            outs=[a2a_out[i][:].opt()])
                    if box["g"] is not None:
                        drain(box["g"])
                # proj(C1) doubles as keep-warm cover for the C2 transfer
                # window; a few scratch matmuls bridge any remaining gap so
                # proj(C2) starts at the warm clock.
                drain(proj_units(1, slabs[1]))
                for _ in range(4):
                    ps = qps.tile([128, QI], F32, tag="ps", name="warm")
                    for dd in range(DT):
                        nc.tensor.matmul(
                            ps[:], w_sb[:, 0, dd], wo_sb[:, dd, 0:QI],
                            start=(dd == 0), stop=(dd == DT - 1))
                drain(proj_units(2))
    nc.finalize()
    return nc


def make_in_maps(states, Wq, Wk, Wv, Wo, n_cores=N_CORES):
    b_, s_, d_ = states.shape
    R = b_ * s_
    HPC = d_ // n_cores
    bf = ml_dtypes.bfloat16
    statesT = np.ascontiguousarray(
        np.asarray(states, dtype=np.float32).reshape(R, d_).T).astype(bf)
    Wq = np.asarray(Wq, dtype=np.float32).astype(bf)
    Wk = np.asarray(Wk, dtype=np.float32).astype(bf)
    Wv = np.asarray(Wv, dtype=np.float32).astype(bf)
    Wo = np.ascontiguousarray(np.asarray(Wo, dtype=np.float32)).astype(bf)
    tri = build_tri()
    in_maps = []
    for c in range(n_cores):
        in_maps.append({
            "statesT": statesT,
            "wq": np.ascontiguousarray(Wq[:, c * HPC:(c + 1) * HPC]),
            "wk": np.ascontiguousarray(Wk[:, c * HPC:(c + 1) * HPC]),
            "wv": np.ascontiguousarray(Wv[:, c * HPC:(c + 1) * HPC]),
            "wo": Wo,
            "tri": tri,
        })
    return in_maps


def unshard(outs, b_, s_, d_, n_cores=N_CORES):
    """Core j's output rows: [C0: rows 512j..][C1: 4096+256j..][C2: ...]."""
    R = b_ * s_
    full = np.empty((R, d_), dtype=np.float32)
    crows = [len(cb) * s_ // n_cores for cb in CHUNK_BATCHES]
    bases = [0, 4096, 6144]
    for j in range(n_cores):
        o = 0
        for chunk in range(3):
            cr = crows[chunk]
            full[bases[chunk] + j * cr: bases[chunk] + (j + 1) * cr] = \
                outs[j][o:o + cr]
            o += cr
    return full.reshape(b_, s_, d_)


_NC_CACHE = {}


def kernel(states, mask, Wq, Wk, Wv, Wo):
    """Full inputs -> full output [B, S, D]. mask is causal by construction
    (reference builds tril); causality is hardcoded on-chip."""
    from concourse.bass_utils import run_bass_kernel_spmd

    states = np.asarray(states, dtype=np.float32)
    b_, s_, d_ = states.shape
    key = (b_, s_, d_)
    if key not in _NC_CACHE:
        _NC_CACHE[key] = build(b_, s_, d_)
    nc = _NC_CACHE[key]
    in_maps = make_in_maps(states, Wq, Wk, Wv, Wo)
    res = run_bass_kernel_spmd(nc, in_maps, core_ids=list(range(N_CORES)))
    outs = [res.results[c]["out"] for c in range(N_CORES)]
    return unshard(outs, b_, s_, d_).astype(np.float32)


# revision 21
# speedup vs baseline: 1.0139x; 1.0139x over previous
"""Distributed attention layer kernel for 8 TRN2 NeuronCores.

Math (per reference): out = softmax_causal((x@Wq)(x@Wk)^T / 8) @ (x@Wv) @ Wo
with B=4, S=2048, D=1024, H=16 heads of dim 64.

Sharding: head tensor-parallel. Core c owns head pair (2c, 2c+1):
  - Wq/Wk/Wv column-sharded [1024, 128]; statesT replicated [1024, 8192].
  - Each core computes qT/kT/vT for its 2 heads, causal attention in
    S^T layout (kj on partitions, qi on free), softmax denominator via a
    ones-column appended to V (PV matmul row 64 = sum of probs).
  - ctx^T tiles are normalized straight out of PSUM: the raw denominator
    row is partition-broadcast on GpSimd (idle engine), inverted in one
    reciprocal_approx_fast, and a single DVE mul (PSUM x SBUF -> bf16)
    writes the AllToAll staging tile. This keeps the in-order DVE queue
    shallow so diag mask-muls never stall PV matmuls.
  - Three AllToAlls: C0 = batches {0,1} (fires after b1, transfer hidden
    under b2's attention), C1 = batch {2} (hidden under b3), C2 = batch
    {3} (small 0.25MB tail, covered by proj(C1) as keep-warm work).
  - Output projection: out_rows = sum_c slab_c.T @ Wo[128c:...] in PSUM.
    proj(C0) interleaves into b3's attention as PE filler; proj(C1)
    drains right after the C2 trigger; proj(C2) is the only exposed tail.

Scheduling: the PE instruction stream is kept dense to hold the clock
at the top p-state. QKV for batch b+1 is interleaved as filler between
the score/PV matmul pairs of batch b's attention. Causal masking of
diagonal blocks is a post-exp multiply by a 0/1 triangular mask (bf16,
SBUF) so the score->exp chain never waits on a PSUM-side DVE add.

Matmul operands are bf16 (PE full rate); accumulation is fp32 in PSUM.
"""

import ml_dtypes
import numpy as np

import concourse.bass as bass
import concourse.mybir as mybir
import concourse.tile as tile
from concourse import bacc
from concourse.masks import make_identity

F32 = mybir.dt.float32
BF16 = mybir.dt.bfloat16

B, S, D, H = 4, 2048, 1024, 16
HD = 64
N_CORES = 8
QI, KJ = 512, 128

# output row chunks per collective: C0 = batches {0,1}, C1 = {2}, C2 = {3}
CHUNK_BATCHES = ((0, 1), (2,), (3,))


def build_tri(KJ=KJ):
    """tri[p, f] = 1.0 if p <= f else 0.0 (valid causal positions of the
    first KJ columns of a diagonal strip)."""
    p = np.arange(KJ)[:, None]
    f = np.arange(KJ)[None, :]
    return np.where(p <= f, 1.0, 0.0).astype(ml_dtypes.bfloat16)


def build(b_=B, s_=S, d_=D, n_cores=N_CORES):
    HPC = d_ // n_cores          # head cols per core (2 heads x 64)
    NH = HPC // HD               # heads per core (2)
    R = b_ * s_                  # global rows (8192)
    Rc = R // n_cores            # output rows per core (1024)
    DT = d_ // 128               # contraction tiles (8)
    SKJ = s_ // KJ               # kj blocks per (b, h) (16)
    SQI = s_ // QI               # qi tiles per (b, h) (4)
    CL = s_ // QI                # column tiles per batch (4)
    # rows per core per chunk
    CROWS = [len(cb) * s_ // n_cores for cb in CHUNK_BATCHES]  # 512, 256, 256
    assert s_ % QI == 0 and d_ % 128 == 0

    nc = bacc.Bacc(None, target_bir_lowering=False, debug=False)
    statesT = nc.declare_dram_parameter("statesT", [d_, R], BF16, isOutput=False)
    wq = nc.declare_dram_parameter("wq", [d_, HPC], BF16, isOutput=False)
    wk = nc.declare_dram_parameter("wk", [d_, HPC], BF16, isOutput=False)
    wv = nc.declare_dram_parameter("wv", [d_, HPC], BF16, isOutput=False)
    wo = nc.declare_dram_parameter("wo", [d_, d_], BF16, isOutput=False)
    tri_in = nc.declare_dram_parameter("tri", [KJ, KJ], BF16, isOutput=False)
    out_ext = nc.declare_dram_parameter("out", [Rc, d_], F32, isOutput=True)

    SC = float(1.0 / np.sqrt(HD))
    EXP = mybir.ActivationFunctionType.Exp

    with tile.TileContext(nc) as tc:
        with tc.tile_pool(name="persist", bufs=1) as pp, \
             tc.tile_pool(name="dram", bufs=1, space="DRAM") as dram:
            a2a_in = [dram.tile([n_cores * HPC, CROWS[i]], BF16,
                                tag=f"a2a_in{i}", name=f"a2a_in{i}")
                      for i in range(3)]
            a2a_out = [dram.tile([n_cores * HPC, CROWS[i]], BF16,
                                 tag=f"a2a_out{i}", name=f"a2a_out{i}")
                       for i in range(3)]

            qT = pp.tile([HPC, R], BF16, tag="qT")
            kT = pp.tile([HPC, R], BF16, tag="kT")
            vp = pp.tile([KJ, b_ * NH * SKJ, HD + 1], BF16, tag="vp")
            w_sb = pp.tile([128, 3, DT, HPC], BF16, tag="w_sb")
            wo_sb = pp.tile([128, DT, d_], BF16, tag="wo_sb")
            tri_sb = pp.tile([KJ, KJ], BF16, tag="tri_sb")
            ident = pp.tile([128, 128], BF16, tag="ident")

            with tc.tile_pool(name="st_in", bufs=4) as stp, \
                 tc.tile_pool(name="vT_pool", bufs=2) as vtp, \
                 tc.tile_pool(name="ps_ps", bufs=2, space="PSUM") as qps, \
                 tc.tile_pool(name="sp_ps", bufs=2, space="PSUM") as spp, \
                 tc.tile_pool(name="ctx_ps", bufs=2, space="PSUM") as cps, \
                 tc.tile_pool(name="pt_sb", bufs=6) as ptp, \
                 tc.tile_pool(name="ctxu_sb", bufs=4) as cup, \
                 tc.tile_pool(name="recip_sb", bufs=3) as rpp, \
                 tc.tile_pool(name="ctxT_sb", bufs=4) as ctp, \
                 tc.tile_pool(name="slab_sb", bufs=2) as slp, \
                 tc.tile_pool(name="o_sb", bufs=3) as osp:

                # ---- prologue: start input DMAs early
                st_tiles = {}

                def issue_st(ci):
                    st = stp.tile([128, DT, QI], BF16, tag="st", name="st")
                    for dd in range(DT):
                        nc.sync.dma_start(
                            out=st[:, dd],
                            in_=statesT[dd * 128:(dd + 1) * 128,
                                        ci * QI:(ci + 1) * QI])
                    st_tiles[ci] = st

                issue_st(0)
                issue_st(1)
                issue_st(2)
                nc.sync.dma_start(out=tri_sb[:], in_=tri_in[:, :])
                for i, w in enumerate([wq, wk, wv]):
                    nc.sync.dma_start(
                        out=w_sb[:, i], in_=w[:, :].rearrange("(t p) c -> p t c", p=128))
                nc.sync.dma_start(
                    out=wo_sb[:], in_=wo[:, :].rearrange("(t p) n -> p t n", p=128))
                make_identity(nc, ident[:])
                nc.vector.memset(vp[:, :, HD], 1.0)

                def qkv_units(bb):
                    """Yield-granular QKV + V' transposes for batch bb."""
                    vT = vtp.tile([HPC, s_], BF16, tag="vT", name="vT")
                    for cl in range(CL):
                        ci = bb * CL + cl
                        if ci + 3 < b_ * CL:
                            issue_st(ci + 3)
                        st = st_tiles.pop(ci)
                        yield
                        for pi, dest, off in ((2, vT, cl * QI), (0, qT, ci * QI),
                                              (1, kT, ci * QI)):
                            ps = qps.tile([128, QI], F32, tag="ps", name="ps")
                            for dd in range(DT):
                                nc.tensor.matmul(
                                    ps[:], w_sb[:, pi, dd], st[:, dd],
                                    start=(dd == 0), stop=(dd == DT - 1))
                                if dd % 2 == 1:
                                    yield
                            nc.vector.tensor_copy(dest[:, off:off + QI], ps[:])
                            yield
                        # h-inner: consecutive transposes load alternating
                        # row groups (base partition 0/64), so each LDW pulls
                        # ahead under the other head's in-flight transpose
                        for kj in range(cl * (SKJ // CL), (cl + 1) * (SKJ // CL)):
                            for h in range(NH):
                                blk = (bb * NH + h) * SKJ + kj
                                tp = qps.tile([KJ, HD], BF16, tag="ps", name="tp")
                                nc.tensor.transpose(
                                    tp[0:KJ, 0:HD],
                                    vT[h * HD:(h + 1) * HD, kj * KJ:(kj + 1) * KJ],
                                    ident[h * HD:(h + 1) * HD, h * HD:(h + 1) * HD])
                                nc.vector.tensor_copy(vp[:, blk, 0:HD],
                                                      tp[0:KJ, 0:HD])
                                yield

                def dma_ctx_to_a2a(ctxT, bb, qi, hh):
                    """Stage a normalized ctx^T tile into its chunk buffer."""
                    chunk = 0 if bb < 2 else bb - 1
                    r0 = bb * s_ + qi * QI            # global row of tile col 0
                    base = 0 if chunk == 0 else (4096 if chunk == 1 else 6144)
                    crows = CROWS[chunk]
                    # tile cols [c0, c0+crows) -> dest core j, chunk-col offset
                    for part in range(QI // crows):
                        c0 = part * crows
                        j = (r0 + c0 - base) // crows
                        nc.sync.dma_start(
                            out=a2a_in[chunk][j * HPC + hh * HD:
                                              j * HPC + (hh + 1) * HD, 0:crows],
                            in_=ctxT[:, c0:c0 + crows])

                def load_slab(chunk):
                    """Issue the slab DMAs for one chunk. Must be emitted at
                    a point where collective `chunk` is known complete: a
                    DMA descriptor waiting on its semaphore would sit at the
                    queue head and also because the next collective's ring
                    traffic is FIFO-ordered behind it in the same queues."""
                    crows = CROWS[chunk]
                    slab = slp.tile([HPC, n_cores, crows], BF16,
                                    tag=f"slab{chunk}", name="slab", bufs=1)
                    for c in range(n_cores):
                        nc.sync.dma_start(
                            out=slab[:, c],
                            in_=a2a_out[chunk][c * HPC:(c + 1) * HPC, :])
                    return slab

                def proj_units(chunk, slab=None):
                    """Yield-granular output projection for one chunk."""
                    crows = CROWS[chunk]
                    obase = sum(CROWS[:chunk])
                    if slab is None:
                        slab = load_slab(chunk)
                    for m in range(crows // 128):
                        for n in range(d_ // QI):
                            ps = qps.tile([128, QI], F32, tag="ps", name="ops")
                            for c in range(n_cores):
                                nc.tensor.matmul(
                                    ps[:],
                                    slab[:, c, m * 128:(m + 1) * 128],
                                    wo_sb[:, c, n * QI:(n + 1) * QI],
                                    start=(c == 0), stop=(c == n_cores - 1))
                                yield
                            ob = osp.tile([128, QI], F32, tag="ob", name="ob")
                            nc.vector.tensor_copy(ob[:], ps[:])
                            nc.sync.dma_start(
                                out=out_ext[obase + m * 128:
                                            obase + (m + 1) * 128,
                                            n * QI:(n + 1) * QI],
                                in_=ob[:])
                            yield

                # Two-stage epilogue pipeline, serviced once per qi. Stage A
                # pulls the raw denominator row + ctx rows to SBUF (freeing
                # the PSUM bank for the next qi) and kicks the GpSimd
                # partition-broadcast (its ~2us latency is hidden: stage B
                # runs a full qi later). Stage B inverts the broadcast
                # denominator in one fast DVE op and one bf16 mul writes the
                # AllToAll staging tile.
                stage_a = []
                stage_b = []

                def service():
                    while stage_b:
                        ctxu, rbd, bb, qi, hh = stage_b.pop(0)
                        rb = rpp.tile([HD, QI], F32, tag="rb", name="rb",
                                      bufs=4)
                        nc.vector.reciprocal_approx_fast(rb[:], rbd[:])
                        ctxT = ctp.tile([HD, QI], BF16, tag="ctxT",
                                        name="ctxT")
                        nc.vector.tensor_mul(ctxT[:], ctxu[:], rb[:])
                        dma_ctx_to_a2a(ctxT, bb, qi, hh)
                    while stage_a:
                        ctx, bb, qi, hh = stage_a.pop(0)
                        # den must land on partition 0: partition_broadcast
                        # broadcasts partition 0 regardless of the AP offset
                        den0 = rpp.tile([1, QI], F32, tag="den0",
                                        name="den0", bufs=4)
                        nc.vector.tensor_copy(den0[:], ctx[HD:HD + 1, :])
                        ctxu = cup.tile([HD, QI], BF16, tag="ctxu",
                                        name="ctxu", bufs=4)
                        nc.vector.tensor_copy(ctxu[:], ctx[0:HD, :])
                        rbd = rpp.tile([HD, QI], F32, tag="rbd", name="rbd",
                                       bufs=4)
                        nc.gpsimd.partition_broadcast(rbd[:], den0[:])
                        stage_b.append((ctxu, rbd, bb, qi, hh))

                def flush_pending():
                    service()
                    service()

                def attn_pair(bb, fill, pre_qi=None):
                    """Attention for both heads of batch bb, interleaved so
                    the two heads' K=64 score matmuls sit in adjacent row
                    groups of the PE array (tile_position auto-derived from
                    base partition 0/64) and run concurrently; their
                    LDWEIGHTS pull ahead into the other head's row group."""
                    base = bb * s_
                    for qi in range(SQI):
                        if pre_qi is not None:
                            pre_qi(qi)
                        # service point first: frees last qi's ctx banks
                        # (stage-A copies) before this qi's PVs need them
                        service()
                        ctxs = [cps.tile([HD + 1, QI], F32, tag="ctx",
                                         name="ctx") for _ in range(NH)]
                        q0 = base + qi * QI

                        def s_mm(h, out_ap, kj, coff):
                            nc.tensor.matmul(
                                out_ap,
                                kT[h * HD:(h + 1) * HD,
                                   base + kj * KJ: base + (kj + 1) * KJ],
                                qT[h * HD:(h + 1) * HD, q0 + coff: q0 + QI],
                                start=True, stop=True)

                        def pv_mm(h, kj, pt, c0, c1, coff, start, stop):
                            blk = (bb * NH + h) * SKJ + kj
                            nc.tensor.matmul(
                                ctxs[h][:, coff:QI], vp[:, blk],
                                pt[:, c0:c1], start=start, stop=stop)

                        # Diagonal strip: 4 blocks x 2 heads packed into 3
                        # PSUM regions / 3 exps. Layout per region (bank0 |
                        # bank1): A = h0·di0 | h1·di0; B = h0·di1 | h1·di1;
                        # C = h0·di2, h0·di3 | h1·di2, h1·di3.
                        dkj = SQI * qi
                        rA = spp.tile([128, 2 * QI], F32, tag="sp", name="rA")
                        s_mm(0, rA[:, 0:QI], dkj + 0, 0)
                        s_mm(1, rA[:, QI:2 * QI], dkj + 0, 0)
                        pA = ptp.tile([128, 2 * QI], BF16, tag="pt", name="pA")
                        nc.scalar.activation(pA[:], rA[:], EXP, scale=SC)
                        nc.vector.tensor_mul(pA[:, 0:KJ], pA[:, 0:KJ], tri_sb[:])
                        nc.vector.tensor_mul(pA[:, QI:QI + KJ],
                                             pA[:, QI:QI + KJ], tri_sb[:])
                        fill(2)
                        rB = spp.tile([128, 2 * QI], F32, tag="sp", name="rB")
                        s_mm(0, rB[:, 0:QI - KJ], dkj + 1, KJ)
                        s_mm(1, rB[:, QI:2 * QI - KJ], dkj + 1, KJ)
                        pv_mm(0, dkj + 0, pA, 0, QI, 0, True, False)
                        pv_mm(1, dkj + 0, pA, QI, 2 * QI, 0, True, False)
                        fill(1)
                        pB = ptp.tile([128, 2 * QI], BF16, tag="pt", name="pB")
                        nc.scalar.activation(pB[:, 0:2 * QI - KJ],
                                             rB[:, 0:2 * QI - KJ], EXP,
                                             scale=SC)
                        nc.vector.tensor_mul(pB[:, 0:KJ], pB[:, 0:KJ], tri_sb[:])
                        nc.vector.tensor_mul(pB[:, QI:QI + KJ],
                                             pB[:, QI:QI + KJ], tri_sb[:])
                        fill(1)
                        rC = spp.tile([128, 2 * QI], F32, tag="sp", name="rC")
                        s_mm(0, rC[:, 0:2 * KJ], dkj + 2, 2 * KJ)
                        s_mm(1, rC[:, QI:QI + 2 * KJ], dkj + 2, 2 * KJ)
                        s_mm(0, rC[:, 2 * KJ:3 * KJ], dkj + 3, 3 * KJ)
                        s_mm(1, rC[:, QI + 2 * KJ:QI + 3 * KJ], dkj + 3, 3 * KJ)
                        pv_mm(0, dkj + 1, pB, 0, QI - KJ, KJ, False, False)
                        pv_mm(1, dkj + 1, pB, QI, 2 * QI - KJ, KJ,
                              False, False)
                        fill(1)
                        pC = ptp.tile([128, 2 * QI], BF16, tag="pt", name="pC")
                        nc.scalar.activation(pC[:, 0:QI + 3 * KJ],
                                             rC[:, 0:QI + 3 * KJ], EXP,
                                             scale=SC)
                        for h in range(NH):
                            off = h * QI
                            nc.vector.tensor_mul(pC[:, off:off + KJ],
                                                 pC[:, off:off + KJ],
                                                 tri_sb[:])
                            nc.vector.tensor_mul(
                                pC[:, off + 2 * KJ:off + 3 * KJ],
                                pC[:, off + 2 * KJ:off + 3 * KJ], tri_sb[:])
                        fill(2)
                        last_diag = qi == 0
                        pv_mm(0, dkj + 2, pC, 0, 2 * KJ, 2 * KJ,
                              False, False)
                        pv_mm(1, dkj + 2, pC, QI, QI + 2 * KJ, 2 * KJ,
                              False, False)
                        pv_mm(0, dkj + 3, pC, 2 * KJ, 3 * KJ, 3 * KJ,
                              False, last_diag)
                        pv_mm(1, dkj + 3, pC, QI + 2 * KJ, QI + 3 * KJ,
                              3 * KJ, False, last_diag)
                        fill(1)
                        # full blocks: one kj x both heads per region
                        for kj in range(SQI * qi):
                            reg = spp.tile([128, 2 * QI], F32, tag="sp",
                                           name="reg")
                            s_mm(0, reg[:, 0:QI], kj, 0)
                            s_mm(1, reg[:, QI:2 * QI], kj, 0)
                            pt = ptp.tile([128, 2 * QI], BF16, tag="pt",
                                          name="pt")
                            nc.scalar.activation(pt[:], reg[:], EXP, scale=SC)
                            fill(2)
                            stop = kj == SQI * qi - 1
                            pv_mm(0, kj, pt, 0, QI, 0, False, stop)
                            pv_mm(1, kj, pt, QI, 2 * QI, 0, False, stop)
                            fill(1)
                        for h in range(NH):
                            stage_a.append((ctxs[h], bb, qi, h))
                        fill(1)

                def drain(gen):
                    for _ in gen:
                        pass

                def make_fill(gen):
                    box = {"g": gen}

                    def fill(n=1):
                        g = box["g"]
                        if g is None:
                            return
                        for _ in range(n):
                            try:
                                next(g)
                            except StopIteration:
                                box["g"] = None
                                return
                    return fill, box

                slabs = {}

                def pre_qi_b3(qi):
                    # C1 completed early in b3's attention; issue its slab
                    # loads before C2's collective ring traffic (FIFO DMA
                    # queues) can get ahead of them.
                    if qi == SQI - 1:
                        slabs[1] = load_slab(1)

                drain(qkv_units(0))
                for bb in range(b_):
                    if bb < b_ - 1:
                        gen = qkv_units(bb + 1)
                    else:
                        # C0 completed during b2's attention; its slab is
                        # ready, so proj(C0) interleaves as real PE filler.
                        gen = proj_units(0)
                    fill, box = make_fill(gen)
                    attn_pair(bb, fill, pre_qi_b3 if bb == b_ - 1 else None)
                    if bb >= 1:
                        flush_pending()
                        i = bb - 1
                        nc.gpsimd.collective_compute(
                            "AllToAll", mybir.AluOpType.bypass,
                            replica_groups=[list(range(n_cores))],
                            ins=[a2a_in[i][:].opt()],
                            outs=[a2a_out[i][:].opt()])
                    if box["g"] is not None:
                        drain(box["g"])
                # proj(C1) doubles as keep-warm cover for the C2 transfer
                # window; a few scratch matmuls bridge any remaining gap so
                # proj(C2) starts at the warm clock.
                drain(proj_units(1, slabs[1]))
                for _ in range(4):
                    ps = qps.tile([128, QI], F32, tag="ps", name="warm")
                    for dd in range(DT):
                        nc.tensor.matmul(
                            ps[:], w_sb[:, 0, dd], wo_sb[:, dd, 0:QI],
                            start=(dd == 0), stop=(dd == DT - 1))
                drain(proj_units(2))
    nc.finalize()
    return nc


def make_in_maps(states, Wq, Wk, Wv, Wo, n_cores=N_CORES):
    b_, s_, d_ = states.shape
    R = b_ * s_
    HPC = d_ // n_cores
    bf = ml_dtypes.bfloat16
    statesT = np.ascontiguousarray(
        np.asarray(states, dtype=np.float32).reshape(R, d_).T).astype(bf)
    Wq = np.asarray(Wq, dtype=np.float32).astype(bf)
    Wk = np.asarray(Wk, dtype=np.float32).astype(bf)
    Wv = np.asarray(Wv, dtype=np.float32).astype(bf)
    Wo = np.ascontiguousarray(np.asarray(Wo, dtype=np.float32)).astype(bf)
    tri = build_tri()
    in_maps = []
    for c in range(n_cores):
        in_maps.append({
            "statesT": statesT,
            "wq": np.ascontiguousarray(Wq[:, c * HPC:(c + 1) * HPC]),
            "wk": np.ascontiguousarray(Wk[:, c * HPC:(c + 1) * HPC]),
            "wv": np.ascontiguousarray(Wv[:, c * HPC:(c + 1) * HPC]),
            "wo": Wo,
            "tri": tri,
        })
    return in_maps


def unshard(outs, b_, s_, d_, n_cores=N_CORES):
    """Core j's output rows: [C0: rows 512j..][C1: 4096+256j..][C2: ...]."""
    R = b_ * s_
    full = np.empty((R, d_), dtype=np.float32)
    crows = [len(cb) * s_ // n_cores for cb in CHUNK_BATCHES]
    bases = [0, 4096, 6144]
    for j in range(n_cores):
        o = 0
        for chunk in range(3):
            cr = crows[chunk]
            full[bases[chunk] + j * cr: bases[chunk] + (j + 1) * cr] = \
                outs[j][o:o + cr]
            o += cr
    return full.reshape(b_, s_, d_)


_NC_CACHE = {}


def kernel(states, mask, Wq, Wk, Wv, Wo):
    """Full inputs -> full output [B, S, D]. mask is causal by construction
    (reference builds tril); causality is hardcoded on-chip."""
    from concourse.bass_utils import run_bass_kernel_spmd

    states = np.asarray(states, dtype=np.float32)
    b_, s_, d_ = states.shape
    key = (b_, s_, d_)
    if key not in _NC_CACHE:
        _NC_CACHE[key] = build(b_, s_, d_)
    nc = _NC_CACHE[key]
    in_maps = make_in_maps(states, Wq, Wk, Wv, Wo)
    res = run_bass_kernel_spmd(nc, in_maps, core_ids=list(range(N_CORES)))
    outs = [res.results[c]["out"] for c in range(N_CORES)]
    return unshard(outs, b_, s_, d_).astype(np.float32)
